# revision 31
# baseline (speedup 1.0000x reference)
"""Multi-head attention kernel for 8 TRN2 NeuronCores (Bass/Tile).

Problem: x[2,2048,1024], 16 heads x 64 dims, torch-style Linear weights.
Sharding: data parallel over batch (2) x tensor parallel over heads (16/4):
core c handles batch c//4, heads 4*(c%4) .. 4*(c%4)+3. Each core computes
its heads' attention output projected through its slice of wo, producing a
partial [2048, 1024] fp16 output; the host sums the 4 partials per batch
(the "all-reduce after wo").

Device dataflow per core (matmul operands bf16, fp32 accumulation):
  QT/KT = weight-slice projections in [d, t] layout (d on partitions)
  V     = projection in natural [s, d] layout, with a ones column appended
          per head so the P@V matmul also yields the softmax denominator
  S^T   = K^T.T @ Q^T per head ([s, t] layout, s on partitions)
  P^T   = exp(S^T / 8) via ScalarE (no max subtraction: logits are O(8))
  O^T   = V.T @ P^T accumulated over s in PSUM (row 64 = denominator)
  y     = (O^T / denom)^T @ wo-slice^T, partial over this core's heads

The kernel is PE-bound at ~94% TensorE occupancy (~880 matmuls at
~215ns issue pitch); the 128-ACTIVATE exp train (~1.11us each) runs
inside that envelope.  Structure:
  - the exp table set is preloaded via a dummy ACTIVATE at t=0
  - input DMA order is wk, wq, x-first-half, wv, x-second-half, wo; the
    chase computes K-chunk0-g0 + Q(tb0) against the landing chunks and
    the ACT train starts as soon as the first half of x is projected
    (slots 0-7 only touch s<1024)
  - everything else (V, remaining K/Q groups, normalization, final wo
    matmuls, y DMAs) is emitted by a build-time budget scheduler:
    EDF-ordered micro-closures fill each slot's spare PE time, and the
    P@V consumers lag elastically behind the ACT train (deep pt
    buffering) gated on their V tile being ready
  - y is written as fp16 partials (summed on host), staged per 128-row
    block and DMA'd with 2KB descriptors as soon as each block's casts
    land; tail copies for the last pair run on the otherwise-idle
    ScalarE

HW-validated pitfalls baked into this design: a PSUM accumulation group
owns its whole 2KB bank (never interleave two groups in one bank); DVE
ops reading PSUM operands directly (reciprocal/tensor_mul) silently
corrupt on HW though CoreSim accepts them -- always stage through SBUF.
"""

import sys

sys.path.insert(0, "/opt/trn_rl_repo")

from contextlib import ExitStack

import ml_dtypes
import numpy as np

import concourse.bass as bass
import concourse.tile as tile
from concourse import bacc, mybir
from concourse import bass_utils
from concourse.bass_interp import get_hw_module

BF16 = mybir.dt.bfloat16
F16 = mybir.dt.float16
F32 = mybir.dt.float32
EXP = mybir.ActivationFunctionType.Exp

N_EMBD = 1024
N_HEAD = 16
HEAD_DIM = 64

N_CORES = 8
HEADS_PER_CORE = 4
DH = HEADS_PER_CORE * HEAD_DIM  # 256


def build_program(T=2048, C=N_EMBD, enable_asserts=False):
    nc = bacc.Bacc(
        "TRN2", target_bir_lowering=False, debug=False, enable_asserts=enable_asserts
    )

    xT = nc.dram_tensor("xT", [C, T], BF16, kind="ExternalInput").ap()
    wqT = nc.dram_tensor("wqT", [C, DH], BF16, kind="ExternalInput").ap()
    wkT = nc.dram_tensor("wkT", [C, DH], BF16, kind="ExternalInput").ap()
    wvT = nc.dram_tensor("wvT", [C, DH], BF16, kind="ExternalInput").ap()
    woT = nc.dram_tensor("woT", [DH, C], BF16, kind="ExternalInput").ap()
    y = nc.dram_tensor("y", [T, C], F16, kind="ExternalOutput").ap()

    n_ct = C // 128   # contraction tiles over embedding dim
    n_st = T // 128   # s tiles (key/value positions)
    n_tb = T // 512   # query blocks
    n_cb = C // 512   # output column blocks

    scale = float(HEAD_DIM**-0.5)

    with tile.TileContext(nc) as tc, ExitStack() as ctx:
        statics = ctx.enter_context(tc.tile_pool(name="statics", bufs=1))
        pt_pool = ctx.enter_context(tc.tile_pool(name="pt", bufs=22))
        onorm_pool = ctx.enter_context(tc.tile_pool(name="onorm", bufs=6))
        small = ctx.enter_context(tc.tile_pool(name="small", bufs=6))
        out_stage = ctx.enter_context(tc.tile_pool(name="out_stage", bufs=4))

        psum_s = ctx.enter_context(tc.tile_pool(name="psum_s", bufs=2, space="PSUM"))
        psum_o = ctx.enter_context(tc.tile_pool(name="psum_o", bufs=2, space="PSUM"))
        psum_f = ctx.enter_context(tc.tile_pool(name="psum_f", bufs=2, space="PSUM"))

        # ---- static SBUF tensors ----
        xT_sb = statics.tile([128, n_ct, T], BF16)
        wq_sb = statics.tile([128, n_ct, DH], BF16)
        wk_sb = statics.tile([128, n_ct, DH], BF16)
        wv_sb = statics.tile([128, n_ct, DH], BF16)
        wo_sb = statics.tile([128, 2, C], BF16)
        qT_sb = statics.tile([128, 2, T], BF16)
        kT_sb = statics.tile([128, 2, T], BF16)
        v_sb = statics.tile([128, n_st, HEADS_PER_CORE, HEAD_DIM + 1], BF16)
        oT_sb = statics.tile([128, 2, T], BF16)
        ones_sb = statics.tile([1, 64], BF16)
        warm_in = statics.tile([1, 64], F32)
        warm_out = statics.tile([1, 64], F32)

        # preload the exp table set while DMAs run (saves ~2.7us later)
        nc.vector.memset(warm_in, 0.0)
        nc.scalar.activation(warm_out, warm_in, EXP)

        xT_chunked = xT.rearrange("(a p) t -> p a t", p=128)
        half = T // 2
        nc.sync.dma_start(out=wk_sb, in_=wkT.rearrange("(a p) d -> p a d", p=128))
        nc.sync.dma_start(out=wq_sb, in_=wqT.rearrange("(a p) d -> p a d", p=128))
        for ct in range(n_ct):
            nc.sync.dma_start(out=xT_sb[:, ct, 0:half], in_=xT_chunked[:, ct, 0:half])
        nc.sync.dma_start(out=wv_sb, in_=wvT.rearrange("(a p) d -> p a d", p=128))
        for ct in range(n_ct):
            nc.sync.dma_start(out=xT_sb[:, ct, half:T], in_=xT_chunked[:, ct, half:T])
        nc.sync.dma_start(out=wo_sb, in_=woT.rearrange("(a p) c -> p a c", p=128))
        nc.vector.memset(ones_sb, 1.0)
        nc.vector.memset(v_sb[:, :, :, HEAD_DIM : HEAD_DIM + 1], 1.0)

        # ---- PE warm-up: tiny dependency-free matmuls keep the PE busy
        # from ~7us through the input-DMA wait, so HAM reaches K=8/8 before
        # the chase starts and never re-throttles into it.
        warm_ps = psum_f.tile([64, 64], F32, tag="f")
        for _ in range(120):
            nc.tensor.matmul(
                warm_ps, lhsT=ones_sb, rhs=ones_sb, start=True, stop=True
            )

        # ---- startup chase: K chunk0 groups 0/1 (s 0-1023) + Q chunk0 tb0,
        # accumulated ct-outer chasing the first-half x chunks as they land.
        # The ACT train starts right after: slots 0-7 only touch s<1024.
        # K g2/g3, Q tb1 and all of V are scheduler tasks (O-pops are
        # elastic, so V tiles may trail their s-slot by many slots).
        proj_scope = nc.named_scope("proj")
        proj_scope.__enter__()

        kps0 = psum_s.tile([128, 1024], F32, tag="s")
        qps0 = psum_f.tile([128, 512], F32, tag="f")
        # K g0 (DMA-paced) then Q tb0; their copies run on ScalarE/DVE in
        # parallel, so S(0) issues ~0.7us after the Q loop ends.  K g1
        # (first read at slot 4) is deferred into the early slot budget.
        for ct in range(n_ct):
            for g in range(2):
                nc.tensor.matmul(
                    kps0[:, g * 512 : (g + 1) * 512],
                    lhsT=wk_sb[:, ct, 0:128],
                    rhs=xT_sb[:, ct, g * 512 : (g + 1) * 512],
                    start=(ct == 0),
                    stop=(ct == n_ct - 1),
                )
            nc.tensor.matmul(
                qps0,
                lhsT=wq_sb[:, ct, 0:128],
                rhs=xT_sb[:, ct, 0:512],
                start=(ct == 0),
                stop=(ct == n_ct - 1),
            )
        nc.scalar.copy(kT_sb[:, 0, 0:512], kps0[:, 0:512])
        nc.vector.tensor_copy(qT_sb[:, 0, 0:512], qps0)
        nc.vector.tensor_copy(kT_sb[:, 0, 512:1024], kps0[:, 512:1024])

        proj_scope.__exit__(None, None, None)

        # ---- attention: software-pipelined head-pair chunk loop ----
        # One slot = one s-tile for a pair of heads (hp): the two K=64 score
        # matmuls land on disjoint PE row groups (base partitions 0 and 64)
        # and run concurrently; both scores share one [128,1024] PSUM tile so
        # a single ACTIVATE computes exp for both heads.
        chunk_list = []
        for hp in range(HEADS_PER_CORE // 2):
            for tb in range(n_tb):
                for c in range(n_st):
                    chunk_list.append((tb, hp, c))
        n_total = len(chunk_list)

        o_ps_map = {}
        denom_map = {}
        o_unnorm_map = {}

        def emit_S_ACT(i):
            tb, hp, st = chunk_list[i]
            chunk_hd = hp
            if st == 0:
                o_psA = psum_o.tile([HEAD_DIM + 1, 512], F32, tag="o")
                o_psB = psum_o.tile([HEAD_DIM + 1, 512], F32, tag="o")
                o_ps_map[(tb, 2 * hp)] = o_psA
                o_ps_map[(tb, 2 * hp + 1)] = o_psB
            with nc.named_scope("S"):
                ps = psum_s.tile([128, 2 * 512], F32, tag="s")
                for half_i, dlo in ((0, 0), (1, 64)):
                    nc.tensor.matmul(
                        ps[:, half_i * 512 : (half_i + 1) * 512],
                        lhsT=kT_sb[dlo : dlo + 64, chunk_hd, st * 128 : (st + 1) * 128],
                        rhs=qT_sb[dlo : dlo + 64, chunk_hd, tb * 512 : (tb + 1) * 512],
                        start=True,
                        stop=True,
                        tile_position=(dlo, 0),
                    )
            with nc.named_scope("exp"):
                pt = pt_pool.tile([128, 2, 512], BF16)
                nc.scalar.activation(
                    pt.rearrange("p a b -> p (a b)"), ps, EXP, scale=scale
                )
            return ps, pt

        def emit_O(i, pt):
            tb, hp, st = chunk_list[i]
            with nc.named_scope("O"):
                for half_i in range(2):
                    h = 2 * hp + half_i
                    nc.tensor.matmul(
                        o_ps_map[(tb, h)],
                        lhsT=v_sb[:, st, h, :],
                        rhs=pt[:, half_i, :],
                        start=(st == 0),
                        stop=(st == n_st - 1),
                    )
            if st == n_st - 1:
                # pair (tb, hp) fully accumulated: drain PSUM immediately so
                # the banks recycle for the next pair (unnormalized O + denom)
                tail = hp == 1 and tb == 3
                for half_i in range(2):
                    h = 2 * hp + half_i
                    o_ps = o_ps_map[(tb, h)]
                    o_unnorm = onorm_pool.tile([64, 512], BF16, tag="ou")
                    if tail and half_i == 1:
                        nc.scalar.copy(o_unnorm, o_ps[0:64, :])
                        denom_f = small.tile([1, 512], F32, tag="denom_f")
                        nc.scalar.copy(denom_f, o_ps[64:65, :])
                    else:
                        nc.vector.tensor_copy(o_unnorm, o_ps[0:64, :])
                        denom_f = small.tile([1, 512], F32, tag="denom_f")
                        nc.vector.tensor_copy(denom_f, o_ps[64:65, :])
                    o_unnorm_map[(tb, h)] = o_unnorm
                    denom_map[(tb, h)] = denom_f
                for half_i in range(2):
                    h = 2 * hp + half_i
                    add_task(
                        cur[0] + 7 + half_i, 300.0,
                        (lambda tb=tb, h=h: run_norm(tb, h)),
                    )

        def emit_norm(tb, h):
            ns = nc.named_scope("norm"); ns.__enter__()
            chunk_hd, dlo = h // 2, (h % 2) * 64
            recip_f = small.tile([1, 512], F32, tag="recip_f")
            nc.vector.reciprocal_approx_fast(recip_f, denom_map[(tb, h)])
            recip = small.tile([1, 512], BF16, tag="recip")
            nc.vector.tensor_copy(recip, recip_f)
            rep = psum_f.tile([128, 512], F32, tag="f")
            nc.tensor.matmul(
                rep[0:64, :], lhsT=ones_sb, rhs=recip, start=True, stop=True
            )
            rep_sb = small.tile([64, 512], BF16, tag="rep")
            nc.vector.tensor_copy(rep_sb, rep[0:64, :])
            nc.vector.tensor_mul(
                oT_sb[dlo : dlo + 64, chunk_hd, tb * 512 : (tb + 1) * 512],
                o_unnorm_map[(tb, h)],
                rep_sb,
            )
            ns.__exit__(None, None, None)

        def qk_group_closures(w_sb, dst, chunk, tb):
            cell = {}

            def mk_mm(c0):
                def f():
                    if "ps" not in cell:
                        ps_qk = psum_f.tile([128, 512], F32, tag="f")
                        cell["ps"] = ps_qk
                    ps_qk = cell["ps"]
                    for ct in range(c0, c0 + 2):
                        nc.tensor.matmul(
                            ps_qk,
                            lhsT=w_sb[:, ct, chunk * 128 : (chunk + 1) * 128],
                            rhs=xT_sb[:, ct, tb * 512 : (tb + 1) * 512],
                            start=(ct == 0),
                            stop=(ct == n_ct - 1),
                        )
                return f

            def cp():
                nc.vector.tensor_copy(
                    dst[:, chunk, tb * 512 : (tb + 1) * 512], cell["ps"]
                )

            return [mk_mm(0), mk_mm(2), mk_mm(4), mk_mm(6), cp]

        def v_pair_closures(p):
            """s-tiles 2p, 2p+1 computed SEQUENTIALLY through one psum bank
            (a PSUM accumulation group owns its whole bank, so the two
            s-tiles' groups must not interleave)."""
            cell = {}

            def mk(k, c0):
                def f():
                    if k != cell.get("k"):
                        ps_v_new = psum_f.tile([128, 512], F32, tag="f")
                        cell["ps"] = ps_v_new
                        cell["k"] = k
                    ps_v = cell["ps"]
                    st = 2 * p + k
                    for ct in range(c0, c0 + 4):
                        nc.tensor.matmul(
                            ps_v[:, 0:DH],
                            lhsT=xT_sb[:, ct, st * 128 : (st + 1) * 128],
                            rhs=wv_sb[:, ct, :],
                            start=(ct == 0),
                            stop=(ct == n_ct - 1),
                        )
                return f

            def cp(k):
                def f():
                    st = 2 * p + k
                    nc.vector.tensor_copy(
                        v_sb[:, st, :, 0:HEAD_DIM],
                        cell["ps"][:, 0:DH].rearrange(
                            "p (h d) -> p h d", h=HEADS_PER_CORE
                        ),
                    )
                return f

            return [mk(0, 0), mk(0, 4), cp(0), mk(1, 0), mk(1, 4), cp(1)]

        def final_closures(tb):
            # per-tt staging: each 128-row block CASTs into its own staged
            # [128, 1024] fp16 tile and DMAs out as soon as both halves have
            # landed (2KB descriptors; issued early so the write flight
            # overlaps the remaining compute).  For the last-processed block
            # (tb3) the casts alternate onto the idle ScalarE so the tail
            # is not DVE-serial.
            y_blocked = y.rearrange("(b t p) c -> p b t c", b=n_tb, p=128)
            tail = tb == 3
            cells = {}
            out = []
            for tt4 in range(4):
                for cb in range(n_cb):
                    def f(tt4=tt4, cb=cb):
                        ns = nc.named_scope("final")
                        ns.__enter__()
                        if tt4 not in cells:
                            y_st = out_stage.tile([128, C], F16)
                            cells[tt4] = y_st
                        tt = tb * 4 + tt4
                        ps_fin = psum_f.tile([128, 512], F32, tag="f")
                        for chunk in range(2):
                            nc.tensor.matmul(
                                ps_fin,
                                lhsT=oT_sb[:, chunk, tt * 128 : (tt + 1) * 128],
                                rhs=wo_sb[:, chunk, cb * 512 : (cb + 1) * 512],
                                start=(chunk == 0),
                                stop=(chunk == 1),
                            )
                        dst = cells[tt4][:, cb * 512 : (cb + 1) * 512]
                        if tail and cb == 1:
                            nc.scalar.copy(dst, ps_fin)
                        else:
                            nc.vector.tensor_copy(dst, ps_fin)
                        ns.__exit__(None, None, None)

                    out.append(f)

                def dma(tt4=tt4):
                    nc.sync.dma_start(
                        out=y_blocked[:, tb, tt4, :], in_=cells[tt4]
                    )

                out.append(dma)
            return out

        # ---------- build-time budget scheduler ----------
        # The 128-slot ACT train is the pacer (~1.11us/slot).  Each slot
        # emits its S+ACT, then fills ~BUDGET ns of PE work: overdue
        # hard-deadline tasks first, then O-pops (elastic backlog, gated on
        # the V tile they consume) and EDF-ordered deferred closures.
        import heapq
        from collections import deque

        task_q = []  # (deadline, seq, cost_ns, fn)
        _seq = [0]

        def add_task(deadline, cost, fn):
            heapq.heappush(task_q, (deadline, _seq[0], cost, fn))
            _seq[0] += 1

        v_ready = [False] * n_st
        norm_cnt = {}
        cur = [0]

        def run_norm(tb, h):
            emit_norm(tb, h)
            norm_cnt[tb] = norm_cnt.get(tb, 0) + 1
            if norm_cnt[tb] == HEADS_PER_CORE:
                for k, fn in enumerate(final_closures(tb)):
                    add_task(cur[0] + 1 + k, 520.0, fn)

        # V pairs p=0..7; the cp closures flip v_ready for their s-tile
        for p in range(n_st // 2):
            cl = v_pair_closures(p)
            costs = [450.0, 450.0, 80.0, 450.0, 450.0, 80.0]

            def flag(fn, st):
                def g():
                    fn()
                    v_ready[st] = True
                return g

            cl[2] = flag(cl[2], 2 * p)
            cl[5] = flag(cl[5], 2 * p + 1)
            for fn, c in zip(cl, costs):
                add_task(max(1, 2 * p + 10), c, fn)

        qk_costs = [440.0, 440.0, 440.0, 440.0, 80.0]
        # second-half K chunk0 groups + Q chunk0 tb1-3 (hard deadlines:
        # first read at slots 8/12 and 16/32/48)
        for fn, c in zip(qk_group_closures(wk_sb, kT_sb, 0, 2), qk_costs):
            add_task(6, c, fn)
        for fn, c in zip(qk_group_closures(wk_sb, kT_sb, 0, 3), qk_costs):
            add_task(10, c, fn)
        for fn, c in zip(qk_group_closures(wq_sb, qT_sb, 0, 1), qk_costs):
            add_task(14, c, fn)
        for fn, c in zip(qk_group_closures(wq_sb, qT_sb, 0, 2), qk_costs):
            add_task(29, c, fn)
        for fn, c in zip(qk_group_closures(wq_sb, qT_sb, 0, 3), qk_costs):
            add_task(45, c, fn)
        for g in range(n_tb):
            for fn, c in zip(qk_group_closures(wk_sb, kT_sb, 1, g), qk_costs):
                add_task(58 + 4 * g, c, fn)
        for tb in range(n_tb):
            for fn, c in zip(qk_group_closures(wq_sb, qT_sb, 1, tb), qk_costs):
                add_task(61 + 16 * tb, c, fn)

        pending = deque()
        BUDGET = 640.0
        O_COST = 450.0
        PT_MAX = 20  # pt_pool bufs - 2

        def head_eligible():
            return bool(pending) and v_ready[chunk_list[pending[0][0]][2]]

        def pop_one():
            j, jpt = pending.popleft()
            emit_O(j, jpt)

        for i in range(n_total):
            cur[0] = i
            ps, pt = emit_S_ACT(i)
            pending.append((i, pt))
            # overdue hard tasks run regardless of budget
            while task_q and task_q[0][0] <= i:
                _, _, _, fn = heapq.heappop(task_q)
                fn()
            # keep the pt backlog inside the pool
            while len(pending) > PT_MAX and head_eligible():
                pop_one()
            budget = BUDGET
            while budget > 0:
                if task_q and task_q[0][0] <= i + 2:
                    _, _, c, fn = heapq.heappop(task_q)
                    fn()
                    budget -= c
                elif len(pending) > 2 and head_eligible():
                    pop_one()
                    budget -= O_COST
                elif task_q:
                    _, _, c, fn = heapq.heappop(task_q)
                    fn()
                    budget -= c
                else:
                    break

        # drain phase: everything left, V-gated pops first-come-first-served.
        # A dependency-free warm burst keeps HAM at K=8/8 while the DVE
        # drains the last pair (otherwise the tail finals run at half clock).
        tail_warm = psum_f.tile([64, 64], F32, tag="f")
        for _ in range(60):
            nc.tensor.matmul(
                tail_warm, lhsT=ones_sb, rhs=ones_sb, start=True, stop=True
            )
        cur[0] = n_total
        guard = 0
        while pending or task_q:
            guard += 1
            assert guard < 10000, "scheduler wedged"
            if head_eligible():
                pop_one()
            elif task_q:
                _, _, _, fn = heapq.heappop(task_q)
                fn()
            else:
                assert not pending, "pending O with no V path"

    nc.compile()
    return nc


def make_core_inputs(x, wq, wk, wv, wo):
    """Shard + pre-layout the full inputs into 8 per-core input maps."""
    bf = ml_dtypes.bfloat16
    in_maps = []
    for core in range(N_CORES):
        b = core // 4
        g = core % 4
        lo, hi = g * DH, (g + 1) * DH
        in_maps.append(
            {
                "xT": np.ascontiguousarray(x[b].T).astype(bf),
                "wqT": np.ascontiguousarray(wq[lo:hi, :].T).astype(bf),
                "wkT": np.ascontiguousarray(wk[lo:hi, :].T).astype(bf),
                "wvT": np.ascontiguousarray(wv[lo:hi, :].T).astype(bf),
                "woT": np.ascontiguousarray(wo[:, lo:hi].T).astype(bf),
            }
        )
    return in_maps


_PROGRAM_CACHE = {}


def _get_program():
    if "nc" not in _PROGRAM_CACHE:
        nc = build_program()
        nc.m = get_hw_module(nc.m)
        _PROGRAM_CACHE["nc"] = nc
    return _PROGRAM_CACHE["nc"]


def run_sharded(in_maps, trace=False):
    nc = _get_program()
    return bass_utils.run_bass_kernel_spmd(
        nc, in_maps, core_ids=list(range(N_CORES)), trace=trace
    )


def kernel(x, wq, wk, wv, wo):
    x = np.asarray(x, dtype=np.float32)
    wq = np.asarray(wq, dtype=np.float32)
    wk = np.asarray(wk, dtype=np.float32)
    wv = np.asarray(wv, dtype=np.float32)
    wo = np.asarray(wo, dtype=np.float32)

    in_maps = make_core_inputs(x, wq, wk, wv, wo)
    res = run_sharded(in_maps)

    B, T, C = x.shape
    out = np.zeros((B, T, C), dtype=np.float32)
    for core in range(N_CORES):
        out[core // 4] += res.results[core]["y"].astype(np.float32)
    return out


if __name__ == "__main__":
    rng = np.random.default_rng(0)
    x = rng.standard_normal((2, 2048, 1024), dtype=np.float32)
    s = 1.0 / np.sqrt(N_EMBD)
    ws = [rng.standard_normal((1024, 1024), dtype=np.float32) * s for _ in range(4)]
    out = kernel(x, *ws)
    print("out", out.shape, out.dtype, float(np.abs(out).max()))


# revision 32
# speedup vs baseline: 1.0134x; 1.0134x over previous
"""Multi-head attention kernel for 8 TRN2 NeuronCores (Bass/Tile).

Problem: x[2,2048,1024], 16 heads x 64 dims, torch-style Linear weights.
Sharding: data parallel over batch (2) x tensor parallel over heads (16/4):
core c handles batch c//4, heads 4*(c%4) .. 4*(c%4)+3. Each core computes
its heads' attention output projected through its slice of wo, producing a
partial [2048, 1024] fp16 output; the host sums the 4 partials per batch
(the "all-reduce after wo").

Device dataflow per core (matmul operands bf16, fp32 accumulation):
  QT/KT = weight-slice projections in [d, t] layout (d on partitions)
  V     = projection in natural [s, d] layout, with a ones column appended
          per head so the P@V matmul also yields the softmax denominator
  S^T   = K^T.T @ Q^T per head ([s, t] layout, s on partitions)
  P^T   = exp(S^T / 8) via ScalarE (no max subtraction: logits are O(8))
  O^T   = V.T @ P^T accumulated over s in PSUM (row 64 = denominator)
  y     = (O^T / denom)^T @ wo-slice^T, partial over this core's heads

The kernel is PE-bound at ~94% TensorE occupancy (~880 matmuls at
~215ns issue pitch); the 128-ACTIVATE exp train (~1.11us each) runs
inside that envelope.  Structure:
  - the exp table set is preloaded via a dummy ACTIVATE at t=0
  - input DMA order is wk, wq, x-first-half, wv, x-second-half, wo; the
    chase computes K-chunk0-g0 + Q(tb0) against the landing chunks and
    the ACT train starts as soon as the first half of x is projected
    (slots 0-7 only touch s<1024)
  - everything else (V, remaining K/Q groups, normalization, final wo
    matmuls, y DMAs) is emitted by a build-time budget scheduler:
    EDF-ordered micro-closures fill each slot's spare PE time, and the
    P@V consumers lag elastically behind the ACT train (deep pt
    buffering) gated on their V tile being ready
  - y is written as fp16 partials (summed on host), staged per 128-row
    block and DMA'd with 2KB descriptors as soon as each block's casts
    land; tail copies for the last pair run on the otherwise-idle
    ScalarE

HW-validated pitfalls baked into this design: a PSUM accumulation group
owns its whole 2KB bank (never interleave two groups in one bank); DVE
ops reading PSUM operands directly (reciprocal/tensor_mul) silently
corrupt on HW though CoreSim accepts them -- always stage through SBUF.
"""

import sys

sys.path.insert(0, "/opt/trn_rl_repo")

from contextlib import ExitStack

import ml_dtypes
import numpy as np

import concourse.bass as bass
import concourse.tile as tile
from concourse import bacc, mybir
from concourse import bass_utils
from concourse.bass_interp import get_hw_module

BF16 = mybir.dt.bfloat16
F16 = mybir.dt.float16
F32 = mybir.dt.float32
EXP = mybir.ActivationFunctionType.Exp

N_EMBD = 1024
N_HEAD = 16
HEAD_DIM = 64

N_CORES = 8
HEADS_PER_CORE = 4
DH = HEADS_PER_CORE * HEAD_DIM  # 256


def build_program(T=2048, C=N_EMBD, enable_asserts=False):
    nc = bacc.Bacc(
        "TRN2", target_bir_lowering=False, debug=False, enable_asserts=enable_asserts
    )

    xT = nc.dram_tensor("xT", [C, T], BF16, kind="ExternalInput").ap()
    wqT = nc.dram_tensor("wqT", [C, DH], BF16, kind="ExternalInput").ap()
    wkT = nc.dram_tensor("wkT", [C, DH], BF16, kind="ExternalInput").ap()
    wvT = nc.dram_tensor("wvT", [C, DH], BF16, kind="ExternalInput").ap()
    woT = nc.dram_tensor("woT", [DH, C], BF16, kind="ExternalInput").ap()
    y = nc.dram_tensor("y", [T, C], F16, kind="ExternalOutput").ap()

    n_ct = C // 128   # contraction tiles over embedding dim
    n_st = T // 128   # s tiles (key/value positions)
    n_tb = T // 512   # query blocks
    n_cb = C // 512   # output column blocks

    scale = float(HEAD_DIM**-0.5)

    with tile.TileContext(nc) as tc, ExitStack() as ctx:
        statics = ctx.enter_context(tc.tile_pool(name="statics", bufs=1))
        pt_pool = ctx.enter_context(tc.tile_pool(name="pt", bufs=22))
        onorm_pool = ctx.enter_context(tc.tile_pool(name="onorm", bufs=6))
        small = ctx.enter_context(tc.tile_pool(name="small", bufs=6))
        out_stage = ctx.enter_context(tc.tile_pool(name="out_stage", bufs=4))

        psum_s = ctx.enter_context(tc.tile_pool(name="psum_s", bufs=2, space="PSUM"))
        psum_o = ctx.enter_context(tc.tile_pool(name="psum_o", bufs=2, space="PSUM"))
        psum_f = ctx.enter_context(tc.tile_pool(name="psum_f", bufs=2, space="PSUM"))

        # ---- static SBUF tensors ----
        xT_sb = statics.tile([128, n_ct, T], BF16)
        wq_sb = statics.tile([128, n_ct, DH], BF16)
        wk_sb = statics.tile([128, n_ct, DH], BF16)
        wv_sb = statics.tile([128, n_ct, DH], BF16)
        wo_sb = statics.tile([128, 2, C], BF16)
        qT_sb = statics.tile([128, 2, T], BF16)
        kT_sb = statics.tile([128, 2, T], BF16)
        v_sb = statics.tile([128, n_st, HEADS_PER_CORE, HEAD_DIM + 1], BF16)
        oT_sb = statics.tile([128, 2, T], BF16)
        ones_sb = statics.tile([1, 64], BF16)
        warm_in = statics.tile([1, 64], F32)
        warm_out = statics.tile([1, 64], F32)

        # preload the exp table set while DMAs run (saves ~2.7us later)
        nc.vector.memset(warm_in, 0.0)
        nc.scalar.activation(warm_out, warm_in, EXP)

        xT_chunked = xT.rearrange("(a p) t -> p a t", p=128)
        half = T // 2
        nc.sync.dma_start(out=wk_sb, in_=wkT.rearrange("(a p) d -> p a d", p=128))
        nc.sync.dma_start(out=wq_sb, in_=wqT.rearrange("(a p) d -> p a d", p=128))
        for ct in range(n_ct):
            nc.sync.dma_start(out=xT_sb[:, ct, 0:half], in_=xT_chunked[:, ct, 0:half])
        nc.sync.dma_start(out=wv_sb, in_=wvT.rearrange("(a p) d -> p a d", p=128))
        for ct in range(n_ct):
            nc.sync.dma_start(out=xT_sb[:, ct, half:T], in_=xT_chunked[:, ct, half:T])
        nc.sync.dma_start(out=wo_sb, in_=woT.rearrange("(a p) c -> p a c", p=128))
        nc.vector.memset(ones_sb, 1.0)
        nc.vector.memset(v_sb[:, :, :, HEAD_DIM : HEAD_DIM + 1], 1.0)

        # ---- startup chase: K chunk0 groups 0/1 (s 0-1023) + Q chunk0 tb0,
        # accumulated ct-outer chasing the first-half x chunks as they land.
        # The ACT train starts right after: slots 0-7 only touch s<1024.
        # K g2/g3, Q tb1 and all of V are scheduler tasks (O-pops are
        # elastic, so V tiles may trail their s-slot by many slots).
        proj_scope = nc.named_scope("proj")
        proj_scope.__enter__()

        kps0 = psum_s.tile([128, 1024], F32, tag="s")
        qps0 = psum_f.tile([128, 512], F32, tag="f")
        # K g0 (DMA-paced) then Q tb0; their copies run on ScalarE/DVE in
        # parallel, so S(0) issues ~0.7us after the Q loop ends.  K g1
        # (first read at slot 4) is deferred into the early slot budget.
        for ct in range(n_ct):
            for g in range(2):
                nc.tensor.matmul(
                    kps0[:, g * 512 : (g + 1) * 512],
                    lhsT=wk_sb[:, ct, 0:128],
                    rhs=xT_sb[:, ct, g * 512 : (g + 1) * 512],
                    start=(ct == 0),
                    stop=(ct == n_ct - 1),
                )
            nc.tensor.matmul(
                qps0,
                lhsT=wq_sb[:, ct, 0:128],
                rhs=xT_sb[:, ct, 0:512],
                start=(ct == 0),
                stop=(ct == n_ct - 1),
            )
        nc.scalar.copy(kT_sb[:, 0, 0:512], kps0[:, 0:512])
        nc.vector.tensor_copy(qT_sb[:, 0, 0:512], qps0)
        nc.vector.tensor_copy(kT_sb[:, 0, 512:1024], kps0[:, 512:1024])

        proj_scope.__exit__(None, None, None)

        # ---- attention: software-pipelined head-pair chunk loop ----
        # One slot = one s-tile for a pair of heads (hp): the two K=64 score
        # matmuls land on disjoint PE row groups (base partitions 0 and 64)
        # and run concurrently; both scores share one [128,1024] PSUM tile so
        # a single ACTIVATE computes exp for both heads.
        chunk_list = []
        for hp in range(HEADS_PER_CORE // 2):
            for tb in range(n_tb):
                for c in range(n_st):
                    chunk_list.append((tb, hp, c))
        n_total = len(chunk_list)

        o_ps_map = {}
        denom_map = {}
        o_unnorm_map = {}

        def emit_S_ACT(i):
            tb, hp, st = chunk_list[i]
            chunk_hd = hp
            if st == 0:
                o_psA = psum_o.tile([HEAD_DIM + 1, 512], F32, tag="o")
                o_psB = psum_o.tile([HEAD_DIM + 1, 512], F32, tag="o")
                o_ps_map[(tb, 2 * hp)] = o_psA
                o_ps_map[(tb, 2 * hp + 1)] = o_psB
            with nc.named_scope("S"):
                ps = psum_s.tile([128, 2 * 512], F32, tag="s")
                for half_i, dlo in ((0, 0), (1, 64)):
                    nc.tensor.matmul(
                        ps[:, half_i * 512 : (half_i + 1) * 512],
                        lhsT=kT_sb[dlo : dlo + 64, chunk_hd, st * 128 : (st + 1) * 128],
                        rhs=qT_sb[dlo : dlo + 64, chunk_hd, tb * 512 : (tb + 1) * 512],
                        start=True,
                        stop=True,
                        tile_position=(dlo, 0),
                    )
            with nc.named_scope("exp"):
                pt = pt_pool.tile([128, 2, 512], BF16)
                nc.scalar.activation(
                    pt.rearrange("p a b -> p (a b)"), ps, EXP, scale=scale
                )
            return ps, pt

        def emit_O(i, pt):
            tb, hp, st = chunk_list[i]
            with nc.named_scope("O"):
                for half_i in range(2):
                    h = 2 * hp + half_i
                    nc.tensor.matmul(
                        o_ps_map[(tb, h)],
                        lhsT=v_sb[:, st, h, :],
                        rhs=pt[:, half_i, :],
                        start=(st == 0),
                        stop=(st == n_st - 1),
                    )
            if st == n_st - 1:
                # pair (tb, hp) fully accumulated: drain PSUM immediately so
                # the banks recycle for the next pair (unnormalized O + denom)
                tail = hp == 1 and tb == 3
                for half_i in range(2):
                    h = 2 * hp + half_i
                    o_ps = o_ps_map[(tb, h)]
                    o_unnorm = onorm_pool.tile([64, 512], BF16, tag="ou")
                    if tail and half_i == 1:
                        nc.scalar.copy(o_unnorm, o_ps[0:64, :])
                        denom_f = small.tile([1, 512], F32, tag="denom_f")
                        nc.scalar.copy(denom_f, o_ps[64:65, :])
                    else:
                        nc.vector.tensor_copy(o_unnorm, o_ps[0:64, :])
                        denom_f = small.tile([1, 512], F32, tag="denom_f")
                        nc.vector.tensor_copy(denom_f, o_ps[64:65, :])
                    o_unnorm_map[(tb, h)] = o_unnorm
                    denom_map[(tb, h)] = denom_f
                for half_i in range(2):
                    h = 2 * hp + half_i
                    add_task(
                        cur[0] + 7 + half_i, 300.0,
                        (lambda tb=tb, h=h: run_norm(tb, h)),
                    )

        def emit_norm(tb, h):
            ns = nc.named_scope("norm"); ns.__enter__()
            chunk_hd, dlo = h // 2, (h % 2) * 64
            recip_f = small.tile([1, 512], F32, tag="recip_f")
            nc.vector.reciprocal_approx_fast(recip_f, denom_map[(tb, h)])
            recip = small.tile([1, 512], BF16, tag="recip")
            nc.vector.tensor_copy(recip, recip_f)
            rep = psum_f.tile([128, 512], F32, tag="f")
            nc.tensor.matmul(
                rep[0:64, :], lhsT=ones_sb, rhs=recip, start=True, stop=True
            )
            rep_sb = small.tile([64, 512], BF16, tag="rep")
            nc.vector.tensor_copy(rep_sb, rep[0:64, :])
            nc.vector.tensor_mul(
                oT_sb[dlo : dlo + 64, chunk_hd, tb * 512 : (tb + 1) * 512],
                o_unnorm_map[(tb, h)],
                rep_sb,
            )
            ns.__exit__(None, None, None)

        def qk_group_closures(w_sb, dst, chunk, tb):
            cell = {}

            def mk_mm(c0):
                def f():
                    if "ps" not in cell:
                        ps_qk = psum_f.tile([128, 512], F32, tag="f")
                        cell["ps"] = ps_qk
                    ps_qk = cell["ps"]
                    for ct in range(c0, c0 + 2):
                        nc.tensor.matmul(
                            ps_qk,
                            lhsT=w_sb[:, ct, chunk * 128 : (chunk + 1) * 128],
                            rhs=xT_sb[:, ct, tb * 512 : (tb + 1) * 512],
                            start=(ct == 0),
                            stop=(ct == n_ct - 1),
                        )
                return f

            def cp():
                nc.vector.tensor_copy(
                    dst[:, chunk, tb * 512 : (tb + 1) * 512], cell["ps"]
                )

            return [mk_mm(0), mk_mm(2), mk_mm(4), mk_mm(6), cp]

        def v_pair_closures(p):
            """s-tiles 2p, 2p+1 computed SEQUENTIALLY through one psum bank
            (a PSUM accumulation group owns its whole bank, so the two
            s-tiles' groups must not interleave)."""
            cell = {}

            def mk(k, c0):
                def f():
                    if k != cell.get("k"):
                        ps_v_new = psum_f.tile([128, 512], F32, tag="f")
                        cell["ps"] = ps_v_new
                        cell["k"] = k
                    ps_v = cell["ps"]
                    st = 2 * p + k
                    for ct in range(c0, c0 + 4):
                        nc.tensor.matmul(
                            ps_v[:, 0:DH],
                            lhsT=xT_sb[:, ct, st * 128 : (st + 1) * 128],
                            rhs=wv_sb[:, ct, :],
                            start=(ct == 0),
                            stop=(ct == n_ct - 1),
                        )
                return f

            def cp(k):
                def f():
                    st = 2 * p + k
                    nc.vector.tensor_copy(
                        v_sb[:, st, :, 0:HEAD_DIM],
                        cell["ps"][:, 0:DH].rearrange(
                            "p (h d) -> p h d", h=HEADS_PER_CORE
                        ),
                    )
                return f

            return [mk(0, 0), mk(0, 4), cp(0), mk(1, 0), mk(1, 4), cp(1)]

        def final_closures(tb):
            # per-tt staging: each 128-row block CASTs into its own staged
            # [128, 1024] fp16 tile and DMAs out as soon as both halves have
            # landed (2KB descriptors; issued early so the write flight
            # overlaps the remaining compute).  For the last-processed block
            # (tb3) the casts alternate onto the idle ScalarE so the tail
            # is not DVE-serial.
            y_blocked = y.rearrange("(b t p) c -> p b t c", b=n_tb, p=128)
            tail = tb == 3
            cells = {}
            out = []
            for tt4 in range(4):
                for cb in range(n_cb):
                    def f(tt4=tt4, cb=cb):
                        ns = nc.named_scope("final")
                        ns.__enter__()
                        if tt4 not in cells:
                            y_st = out_stage.tile([128, C], F16)
                            cells[tt4] = y_st
                        tt = tb * 4 + tt4
                        ps_fin = psum_f.tile([128, 512], F32, tag="f")
                        for chunk in range(2):
                            nc.tensor.matmul(
                                ps_fin,
                                lhsT=oT_sb[:, chunk, tt * 128 : (tt + 1) * 128],
                                rhs=wo_sb[:, chunk, cb * 512 : (cb + 1) * 512],
                                start=(chunk == 0),
                                stop=(chunk == 1),
                            )
                        dst = cells[tt4][:, cb * 512 : (cb + 1) * 512]
                        if tail and cb == 1:
                            nc.scalar.copy(dst, ps_fin)
                        else:
                            nc.vector.tensor_copy(dst, ps_fin)
                        ns.__exit__(None, None, None)

                    out.append(f)

                def dma(tt4=tt4):
                    nc.sync.dma_start(
                        out=y_blocked[:, tb, tt4, :], in_=cells[tt4]
                    )

                out.append(dma)
            return out

        # ---------- build-time budget scheduler ----------
        # The 128-slot ACT train is the pacer (~1.11us/slot).  Each slot
        # emits its S+ACT, then fills ~BUDGET ns of PE work: overdue
        # hard-deadline tasks first, then O-pops (elastic backlog, gated on
        # the V tile they consume) and EDF-ordered deferred closures.
        import heapq
        from collections import deque

        task_q = []  # (deadline, seq, cost_ns, fn)
        _seq = [0]

        def add_task(deadline, cost, fn):
            heapq.heappush(task_q, (deadline, _seq[0], cost, fn))
            _seq[0] += 1

        v_ready = [False] * n_st
        norm_cnt = {}
        cur = [0]

        def run_norm(tb, h):
            emit_norm(tb, h)
            norm_cnt[tb] = norm_cnt.get(tb, 0) + 1
            if norm_cnt[tb] == HEADS_PER_CORE:
                for k, fn in enumerate(final_closures(tb)):
                    add_task(cur[0] + 1 + k, 520.0, fn)

        # V pairs p=0..7; the cp closures flip v_ready for their s-tile
        for p in range(n_st // 2):
            cl = v_pair_closures(p)
            costs = [450.0, 450.0, 80.0, 450.0, 450.0, 80.0]

            def flag(fn, st):
                def g():
                    fn()
                    v_ready[st] = True
                return g

            cl[2] = flag(cl[2], 2 * p)
            cl[5] = flag(cl[5], 2 * p + 1)
            for fn, c in zip(cl, costs):
                add_task(max(1, 2 * p + 10), c, fn)

        qk_costs = [440.0, 440.0, 440.0, 440.0, 80.0]
        # second-half K chunk0 groups + Q chunk0 tb1-3 (hard deadlines:
        # first read at slots 8/12 and 16/32/48)
        for fn, c in zip(qk_group_closures(wk_sb, kT_sb, 0, 2), qk_costs):
            add_task(6, c, fn)
        for fn, c in zip(qk_group_closures(wk_sb, kT_sb, 0, 3), qk_costs):
            add_task(10, c, fn)
        for fn, c in zip(qk_group_closures(wq_sb, qT_sb, 0, 1), qk_costs):
            add_task(14, c, fn)
        for fn, c in zip(qk_group_closures(wq_sb, qT_sb, 0, 2), qk_costs):
            add_task(29, c, fn)
        for fn, c in zip(qk_group_closures(wq_sb, qT_sb, 0, 3), qk_costs):
            add_task(45, c, fn)
        for g in range(n_tb):
            for fn, c in zip(qk_group_closures(wk_sb, kT_sb, 1, g), qk_costs):
                add_task(58 + 4 * g, c, fn)
        for tb in range(n_tb):
            for fn, c in zip(qk_group_closures(wq_sb, qT_sb, 1, tb), qk_costs):
                add_task(61 + 16 * tb, c, fn)

        pending = deque()
        BUDGET = 640.0
        O_COST = 450.0
        PT_MAX = 20  # pt_pool bufs - 2

        def head_eligible():
            return bool(pending) and v_ready[chunk_list[pending[0][0]][2]]

        def pop_one():
            j, jpt = pending.popleft()
            emit_O(j, jpt)

        for i in range(n_total):
            cur[0] = i
            ps, pt = emit_S_ACT(i)
            pending.append((i, pt))
            # overdue hard tasks run regardless of budget
            while task_q and task_q[0][0] <= i:
                _, _, _, fn = heapq.heappop(task_q)
                fn()
            # keep the pt backlog inside the pool
            while len(pending) > PT_MAX and head_eligible():
                pop_one()
            budget = BUDGET
            while budget > 0:
                if task_q and task_q[0][0] <= i + 2:
                    _, _, c, fn = heapq.heappop(task_q)
                    fn()
                    budget -= c
                elif len(pending) > 2 and head_eligible():
                    pop_one()
                    budget -= O_COST
                elif task_q:
                    _, _, c, fn = heapq.heappop(task_q)
                    fn()
                    budget -= c
                else:
                    break

        # drain phase: everything left, V-gated pops first-come-first-served
        cur[0] = n_total
        guard = 0
        while pending or task_q:
            guard += 1
            assert guard < 10000, "scheduler wedged"
            if head_eligible():
                pop_one()
            elif task_q:
                _, _, _, fn = heapq.heappop(task_q)
                fn()
            else:
                assert not pending, "pending O with no V path"

    nc.compile()
    return nc


def make_core_inputs(x, wq, wk, wv, wo):
    """Shard + pre-layout the full inputs into 8 per-core input maps."""
    bf = ml_dtypes.bfloat16
    in_maps = []
    for core in range(N_CORES):
        b = core // 4
        g = core % 4
        lo, hi = g * DH, (g + 1) * DH
        in_maps.append(
            {
                "xT": np.ascontiguousarray(x[b].T).astype(bf),
                "wqT": np.ascontiguousarray(wq[lo:hi, :].T).astype(bf),
                "wkT": np.ascontiguousarray(wk[lo:hi, :].T).astype(bf),
                "wvT": np.ascontiguousarray(wv[lo:hi, :].T).astype(bf),
                "woT": np.ascontiguousarray(wo[:, lo:hi].T).astype(bf),
            }
        )
    return in_maps


_PROGRAM_CACHE = {}


def _get_program():
    if "nc" not in _PROGRAM_CACHE:
        nc = build_program()
        nc.m = get_hw_module(nc.m)
        _PROGRAM_CACHE["nc"] = nc
    return _PROGRAM_CACHE["nc"]


def run_sharded(in_maps, trace=False):
    nc = _get_program()
    return bass_utils.run_bass_kernel_spmd(
        nc, in_maps, core_ids=list(range(N_CORES)), trace=trace
    )


def kernel(x, wq, wk, wv, wo):
    x = np.asarray(x, dtype=np.float32)
    wq = np.asarray(wq, dtype=np.float32)
    wk = np.asarray(wk, dtype=np.float32)
    wv = np.asarray(wv, dtype=np.float32)
    wo = np.asarray(wo, dtype=np.float32)

    in_maps = make_core_inputs(x, wq, wk, wv, wo)
    res = run_sharded(in_maps)

    B, T, C = x.shape
    out = np.zeros((B, T, C), dtype=np.float32)
    for core in range(N_CORES):
        out[core // 4] += res.results[core]["y"].astype(np.float32)
    return out


if __name__ == "__main__":
    rng = np.random.default_rng(0)
    x = rng.standard_normal((2, 2048, 1024), dtype=np.float32)
    s = 1.0 / np.sqrt(N_EMBD)
    ws = [rng.standard_normal((1024, 1024), dtype=np.float32) * s for _ in range(4)]
    out = kernel(x, *ws)
    print("out", out.shape, out.dtype, float(np.abs(out).max()))


# revision 33
# speedup vs baseline: 1.0143x; 1.0009x over previous
"""Multi-head attention kernel for 8 TRN2 NeuronCores (Bass/Tile).

Problem: x[2,2048,1024], 16 heads x 64 dims, torch-style Linear weights.
Sharding: data parallel over batch (2) x tensor parallel over heads (16/4):
core c handles batch c//4, heads 4*(c%4) .. 4*(c%4)+3. Each core computes
its heads' attention output projected through its slice of wo, producing a
partial [2048, 1024] fp16 output; the host sums the 4 partials per batch
(the "all-reduce after wo").

Device dataflow per core (matmul operands bf16, fp32 accumulation):
  QT/KT = weight-slice projections in [d, t] layout (d on partitions)
  V     = projection in natural [s, d] layout, with a ones column appended
          per head so the P@V matmul also yields the softmax denominator
  S^T   = K^T.T @ Q^T per head ([s, t] layout, s on partitions)
  P^T   = exp(S^T / 8) via ScalarE (no max subtraction: logits are O(8))
  O^T   = V.T @ P^T accumulated over s in PSUM (row 64 = denominator)
  y     = (O^T / denom)^T @ wo-slice^T, partial over this core's heads

The kernel is PE-bound at ~94% TensorE occupancy (~880 matmuls at
~215ns issue pitch); the 128-ACTIVATE exp train (~1.11us each) runs
inside that envelope.  Structure:
  - the exp table set is preloaded via a dummy ACTIVATE at t=0
  - input DMA order is wk, wq, x-first-half, wv, x-second-half, wo; the
    chase computes K-chunk0-g0 + Q(tb0) against the landing chunks and
    the ACT train starts as soon as the first half of x is projected
    (slots 0-7 only touch s<1024)
  - everything else (V, remaining K/Q groups, normalization, final wo
    matmuls, y DMAs) is emitted by a build-time budget scheduler:
    EDF-ordered micro-closures fill each slot's spare PE time, and the
    P@V consumers lag elastically behind the ACT train (deep pt
    buffering) gated on their V tile being ready
  - y is written as fp16 partials (summed on host), staged per 128-row
    block and DMA'd with 2KB descriptors as soon as each block's casts
    land; tail copies for the last pair run on the otherwise-idle
    ScalarE

HW-validated pitfalls baked into this design: a PSUM accumulation group
owns its whole 2KB bank (never interleave two groups in one bank); DVE
ops reading PSUM operands directly (reciprocal/tensor_mul) silently
corrupt on HW though CoreSim accepts them -- always stage through SBUF.
"""

import sys

sys.path.insert(0, "/opt/trn_rl_repo")

from contextlib import ExitStack

import ml_dtypes
import numpy as np

import concourse.bass as bass
import concourse.tile as tile
from concourse import bacc, mybir
from concourse import bass_utils
from concourse.bass_interp import get_hw_module

BF16 = mybir.dt.bfloat16
F16 = mybir.dt.float16
F32 = mybir.dt.float32
EXP = mybir.ActivationFunctionType.Exp

N_EMBD = 1024
N_HEAD = 16
HEAD_DIM = 64

N_CORES = 8
HEADS_PER_CORE = 4
DH = HEADS_PER_CORE * HEAD_DIM  # 256


def build_program(T=2048, C=N_EMBD, enable_asserts=False):
    nc = bacc.Bacc(
        "TRN2", target_bir_lowering=False, debug=False, enable_asserts=enable_asserts
    )

    xT = nc.dram_tensor("xT", [C, T], BF16, kind="ExternalInput").ap()
    wqT = nc.dram_tensor("wqT", [C, DH], BF16, kind="ExternalInput").ap()
    wkT = nc.dram_tensor("wkT", [C, DH], BF16, kind="ExternalInput").ap()
    wvT = nc.dram_tensor("wvT", [C, DH], BF16, kind="ExternalInput").ap()
    woT = nc.dram_tensor("woT", [DH, C], BF16, kind="ExternalInput").ap()
    y = nc.dram_tensor("y", [T, C], F16, kind="ExternalOutput").ap()

    n_ct = C // 128   # contraction tiles over embedding dim
    n_st = T // 128   # s tiles (key/value positions)
    n_tb = T // 512   # query blocks
    n_cb = C // 512   # output column blocks

    scale = float(HEAD_DIM**-0.5)

    with tile.TileContext(nc) as tc, ExitStack() as ctx:
        statics = ctx.enter_context(tc.tile_pool(name="statics", bufs=1))
        pt_pool = ctx.enter_context(tc.tile_pool(name="pt", bufs=22))
        onorm_pool = ctx.enter_context(tc.tile_pool(name="onorm", bufs=6))
        small = ctx.enter_context(tc.tile_pool(name="small", bufs=6))
        out_stage = ctx.enter_context(tc.tile_pool(name="out_stage", bufs=4))

        psum_s = ctx.enter_context(tc.tile_pool(name="psum_s", bufs=2, space="PSUM"))
        psum_o = ctx.enter_context(tc.tile_pool(name="psum_o", bufs=2, space="PSUM"))
        psum_f = ctx.enter_context(tc.tile_pool(name="psum_f", bufs=2, space="PSUM"))

        # ---- static SBUF tensors ----
        xT_sb = statics.tile([128, n_ct, T], BF16)
        wq_sb = statics.tile([128, n_ct, DH], BF16)
        wk_sb = statics.tile([128, n_ct, DH], BF16)
        wv_sb = statics.tile([128, n_ct, DH], BF16)
        wo_sb = statics.tile([128, 2, C], BF16)
        qT_sb = statics.tile([128, 2, T], BF16)
        kT_sb = statics.tile([128, 2, T], BF16)
        v_sb = statics.tile([128, n_st, HEADS_PER_CORE, HEAD_DIM + 1], BF16)
        oT_sb = statics.tile([128, 2, T], BF16)
        ones_sb = statics.tile([1, 64], BF16)
        warm_in = statics.tile([1, 64], F32)
        warm_out = statics.tile([1, 64], F32)

        # preload the exp table set while DMAs run (saves ~2.7us later)
        nc.vector.memset(warm_in, 0.0)
        nc.scalar.activation(warm_out, warm_in, EXP)

        xT_chunked = xT.rearrange("(a p) t -> p a t", p=128)
        half = T // 2
        wk_dram = wkT.rearrange("(a p) d -> p a d", p=128)
        wq_dram = wqT.rearrange("(a p) d -> p a d", p=128)
        nc.sync.dma_start(out=wk_sb[:, :, 0:128], in_=wk_dram[:, :, 0:128])
        nc.sync.dma_start(out=wq_sb[:, :, 0:128], in_=wq_dram[:, :, 0:128])
        for ct in range(n_ct):
            nc.sync.dma_start(out=xT_sb[:, ct, 0:half], in_=xT_chunked[:, ct, 0:half])
        nc.sync.dma_start(out=wv_sb, in_=wvT.rearrange("(a p) d -> p a d", p=128))
        for ct in range(n_ct):
            nc.sync.dma_start(out=xT_sb[:, ct, half:T], in_=xT_chunked[:, ct, half:T])
        nc.sync.dma_start(out=wk_sb[:, :, 128:256], in_=wk_dram[:, :, 128:256])
        nc.sync.dma_start(out=wq_sb[:, :, 128:256], in_=wq_dram[:, :, 128:256])
        nc.sync.dma_start(out=wo_sb, in_=woT.rearrange("(a p) c -> p a c", p=128))
        nc.vector.memset(ones_sb, 1.0)
        nc.vector.memset(v_sb[:, :, :, HEAD_DIM : HEAD_DIM + 1], 1.0)

        # ---- startup chase: K chunk0 groups 0/1 (s 0-1023) + Q chunk0 tb0,
        # accumulated ct-outer chasing the first-half x chunks as they land.
        # The ACT train starts right after: slots 0-7 only touch s<1024.
        # K g2/g3, Q tb1 and all of V are scheduler tasks (O-pops are
        # elastic, so V tiles may trail their s-slot by many slots).
        proj_scope = nc.named_scope("proj")
        proj_scope.__enter__()

        kps0 = psum_s.tile([128, 1024], F32, tag="s")
        qps0 = psum_f.tile([128, 512], F32, tag="f")
        # K g0 (DMA-paced) then Q tb0; their copies run on ScalarE/DVE in
        # parallel, so S(0) issues ~0.7us after the Q loop ends.  K g1
        # (first read at slot 4) is deferred into the early slot budget.
        for ct in range(n_ct):
            for g in range(2):
                nc.tensor.matmul(
                    kps0[:, g * 512 : (g + 1) * 512],
                    lhsT=wk_sb[:, ct, 0:128],
                    rhs=xT_sb[:, ct, g * 512 : (g + 1) * 512],
                    start=(ct == 0),
                    stop=(ct == n_ct - 1),
                )
            nc.tensor.matmul(
                qps0,
                lhsT=wq_sb[:, ct, 0:128],
                rhs=xT_sb[:, ct, 0:512],
                start=(ct == 0),
                stop=(ct == n_ct - 1),
            )
        nc.scalar.copy(kT_sb[:, 0, 0:512], kps0[:, 0:512])
        nc.vector.tensor_copy(qT_sb[:, 0, 0:512], qps0)
        nc.vector.tensor_copy(kT_sb[:, 0, 512:1024], kps0[:, 512:1024])

        proj_scope.__exit__(None, None, None)

        # ---- attention: software-pipelined head-pair chunk loop ----
        # One slot = one s-tile for a pair of heads (hp): the two K=64 score
        # matmuls land on disjoint PE row groups (base partitions 0 and 64)
        # and run concurrently; both scores share one [128,1024] PSUM tile so
        # a single ACTIVATE computes exp for both heads.
        chunk_list = []
        for hp in range(HEADS_PER_CORE // 2):
            for tb in range(n_tb):
                for c in range(n_st):
                    chunk_list.append((tb, hp, c))
        n_total = len(chunk_list)

        o_ps_map = {}
        denom_map = {}
        o_unnorm_map = {}

        def emit_S_ACT(i):
            tb, hp, st = chunk_list[i]
            chunk_hd = hp
            if st == 0:
                o_psA = psum_o.tile([HEAD_DIM + 1, 512], F32, tag="o")
                o_psB = psum_o.tile([HEAD_DIM + 1, 512], F32, tag="o")
                o_ps_map[(tb, 2 * hp)] = o_psA
                o_ps_map[(tb, 2 * hp + 1)] = o_psB
            with nc.named_scope("S"):
                ps = psum_s.tile([128, 2 * 512], F32, tag="s")
                for half_i, dlo in ((0, 0), (1, 64)):
                    nc.tensor.matmul(
                        ps[:, half_i * 512 : (half_i + 1) * 512],
                        lhsT=kT_sb[dlo : dlo + 64, chunk_hd, st * 128 : (st + 1) * 128],
                        rhs=qT_sb[dlo : dlo + 64, chunk_hd, tb * 512 : (tb + 1) * 512],
                        start=True,
                        stop=True,
                        tile_position=(dlo, 0),
                    )
            with nc.named_scope("exp"):
                pt = pt_pool.tile([128, 2, 512], BF16)
                nc.scalar.activation(
                    pt.rearrange("p a b -> p (a b)"), ps, EXP, scale=scale
                )
            return ps, pt

        def emit_O(i, pt):
            tb, hp, st = chunk_list[i]
            with nc.named_scope("O"):
                for half_i in range(2):
                    h = 2 * hp + half_i
                    nc.tensor.matmul(
                        o_ps_map[(tb, h)],
                        lhsT=v_sb[:, st, h, :],
                        rhs=pt[:, half_i, :],
                        start=(st == 0),
                        stop=(st == n_st - 1),
                    )
            if st == n_st - 1:
                # pair (tb, hp) fully accumulated: drain PSUM immediately so
                # the banks recycle for the next pair (unnormalized O + denom)
                tail = hp == 1 and tb == 3
                for half_i in range(2):
                    h = 2 * hp + half_i
                    o_ps = o_ps_map[(tb, h)]
                    o_unnorm = onorm_pool.tile([64, 512], BF16, tag="ou")
                    if tail and half_i == 1:
                        nc.scalar.copy(o_unnorm, o_ps[0:64, :])
                        denom_f = small.tile([1, 512], F32, tag="denom_f")
                        nc.scalar.copy(denom_f, o_ps[64:65, :])
                    else:
                        nc.vector.tensor_copy(o_unnorm, o_ps[0:64, :])
                        denom_f = small.tile([1, 512], F32, tag="denom_f")
                        nc.vector.tensor_copy(denom_f, o_ps[64:65, :])
                    o_unnorm_map[(tb, h)] = o_unnorm
                    denom_map[(tb, h)] = denom_f
                for half_i in range(2):
                    h = 2 * hp + half_i
                    add_task(
                        cur[0] + 7 + half_i, 300.0,
                        (lambda tb=tb, h=h: run_norm(tb, h)),
                    )

        def emit_norm(tb, h):
            ns = nc.named_scope("norm"); ns.__enter__()
            tail = tb == 3 and h >= 2
            chunk_hd, dlo = h // 2, (h % 2) * 64
            recip_f = small.tile([1, 512], F32, tag="recip_f")
            nc.vector.reciprocal_approx_fast(recip_f, denom_map[(tb, h)])
            recip = small.tile([1, 512], BF16, tag="recip")
            if tail:
                nc.scalar.copy(recip, recip_f)
            else:
                nc.vector.tensor_copy(recip, recip_f)
            rep = psum_f.tile([128, 512], F32, tag="f")
            nc.tensor.matmul(
                rep[0:64, :], lhsT=ones_sb, rhs=recip, start=True, stop=True
            )
            rep_sb = small.tile([64, 512], BF16, tag="rep")
            if tail:
                nc.scalar.copy(rep_sb, rep[0:64, :])
            else:
                nc.vector.tensor_copy(rep_sb, rep[0:64, :])
            nc.vector.tensor_mul(
                oT_sb[dlo : dlo + 64, chunk_hd, tb * 512 : (tb + 1) * 512],
                o_unnorm_map[(tb, h)],
                rep_sb,
            )
            ns.__exit__(None, None, None)

        def qk_group_closures(w_sb, dst, chunk, tb):
            cell = {}

            def mk_mm(c0):
                def f():
                    if "ps" not in cell:
                        ps_qk = psum_f.tile([128, 512], F32, tag="f")
                        cell["ps"] = ps_qk
                    ps_qk = cell["ps"]
                    for ct in range(c0, c0 + 2):
                        nc.tensor.matmul(
                            ps_qk,
                            lhsT=w_sb[:, ct, chunk * 128 : (chunk + 1) * 128],
                            rhs=xT_sb[:, ct, tb * 512 : (tb + 1) * 512],
                            start=(ct == 0),
                            stop=(ct == n_ct - 1),
                        )
                return f

            def cp():
                nc.vector.tensor_copy(
                    dst[:, chunk, tb * 512 : (tb + 1) * 512], cell["ps"]
                )

            return [mk_mm(0), mk_mm(2), mk_mm(4), mk_mm(6), cp]

        def v_pair_closures(p):
            """s-tiles 2p, 2p+1 computed SEQUENTIALLY through one psum bank
            (a PSUM accumulation group owns its whole bank, so the two
            s-tiles' groups must not interleave)."""
            cell = {}

            def mk(k, c0):
                def f():
                    if k != cell.get("k"):
                        ps_v_new = psum_f.tile([128, 512], F32, tag="f")
                        cell["ps"] = ps_v_new
                        cell["k"] = k
                    ps_v = cell["ps"]
                    st = 2 * p + k
                    for ct in range(c0, c0 + 4):
                        nc.tensor.matmul(
                            ps_v[:, 0:DH],
                            lhsT=xT_sb[:, ct, st * 128 : (st + 1) * 128],
                            rhs=wv_sb[:, ct, :],
                            start=(ct == 0),
                            stop=(ct == n_ct - 1),
                        )
                return f

            def cp(k):
                def f():
                    st = 2 * p + k
                    nc.vector.tensor_copy(
                        v_sb[:, st, :, 0:HEAD_DIM],
                        cell["ps"][:, 0:DH].rearrange(
                            "p (h d) -> p h d", h=HEADS_PER_CORE
                        ),
                    )
                return f

            return [mk(0, 0), mk(0, 4), cp(0), mk(1, 0), mk(1, 4), cp(1)]

        def final_closures(tb):
            # per-tt staging: each 128-row block CASTs into its own staged
            # [128, 1024] fp16 tile and DMAs out as soon as both halves have
            # landed (2KB descriptors; issued early so the write flight
            # overlaps the remaining compute).  For the last-processed block
            # (tb3) the casts alternate onto the idle ScalarE so the tail
            # is not DVE-serial.
            y_blocked = y.rearrange("(b t p) c -> p b t c", b=n_tb, p=128)
            tail = tb == 3
            cells = {}
            out = []
            for tt4 in range(4):
                for cb in range(n_cb):
                    def f(tt4=tt4, cb=cb):
                        ns = nc.named_scope("final")
                        ns.__enter__()
                        if tt4 not in cells:
                            y_st = out_stage.tile([128, C], F16)
                            cells[tt4] = y_st
                        tt = tb * 4 + tt4
                        ps_fin = psum_f.tile([128, 512], F32, tag="f")
                        for chunk in range(2):
                            nc.tensor.matmul(
                                ps_fin,
                                lhsT=oT_sb[:, chunk, tt * 128 : (tt + 1) * 128],
                                rhs=wo_sb[:, chunk, cb * 512 : (cb + 1) * 512],
                                start=(chunk == 0),
                                stop=(chunk == 1),
                            )
                        dst = cells[tt4][:, cb * 512 : (cb + 1) * 512]
                        if tail and cb == 1:
                            nc.scalar.copy(dst, ps_fin)
                        else:
                            nc.vector.tensor_copy(dst, ps_fin)
                        ns.__exit__(None, None, None)

                    out.append(f)

                def dma(tt4=tt4):
                    nc.sync.dma_start(
                        out=y_blocked[:, tb, tt4, :], in_=cells[tt4]
                    )

                out.append(dma)
            return out

        # ---------- build-time budget scheduler ----------
        # The 128-slot ACT train is the pacer (~1.11us/slot).  Each slot
        # emits its S+ACT, then fills ~BUDGET ns of PE work: overdue
        # hard-deadline tasks first, then O-pops (elastic backlog, gated on
        # the V tile they consume) and EDF-ordered deferred closures.
        import heapq
        from collections import deque

        task_q = []  # (deadline, seq, cost_ns, fn)
        _seq = [0]

        def add_task(deadline, cost, fn):
            heapq.heappush(task_q, (deadline, _seq[0], cost, fn))
            _seq[0] += 1

        v_ready = [False] * n_st
        norm_cnt = {}
        cur = [0]

        def run_norm(tb, h):
            emit_norm(tb, h)
            norm_cnt[tb] = norm_cnt.get(tb, 0) + 1
            if norm_cnt[tb] == HEADS_PER_CORE:
                for k, fn in enumerate(final_closures(tb)):
                    add_task(cur[0] + 1 + k, 520.0, fn)

        # V pairs p=0..7; the cp closures flip v_ready for their s-tile
        for p in range(n_st // 2):
            cl = v_pair_closures(p)
            costs = [450.0, 450.0, 80.0, 450.0, 450.0, 80.0]

            def flag(fn, st):
                def g():
                    fn()
                    v_ready[st] = True
                return g

            cl[2] = flag(cl[2], 2 * p)
            cl[5] = flag(cl[5], 2 * p + 1)
            for fn, c in zip(cl, costs):
                add_task(max(1, 2 * p + 10), c, fn)

        qk_costs = [440.0, 440.0, 440.0, 440.0, 80.0]
        # second-half K chunk0 groups + Q chunk0 tb1-3 (hard deadlines:
        # first read at slots 8/12 and 16/32/48)
        for fn, c in zip(qk_group_closures(wk_sb, kT_sb, 0, 2), qk_costs):
            add_task(6, c, fn)
        for fn, c in zip(qk_group_closures(wk_sb, kT_sb, 0, 3), qk_costs):
            add_task(10, c, fn)
        for fn, c in zip(qk_group_closures(wq_sb, qT_sb, 0, 1), qk_costs):
            add_task(14, c, fn)
        for fn, c in zip(qk_group_closures(wq_sb, qT_sb, 0, 2), qk_costs):
            add_task(29, c, fn)
        for fn, c in zip(qk_group_closures(wq_sb, qT_sb, 0, 3), qk_costs):
            add_task(45, c, fn)
        for g in range(n_tb):
            for fn, c in zip(qk_group_closures(wk_sb, kT_sb, 1, g), qk_costs):
                add_task(58 + 4 * g, c, fn)
        for tb in range(n_tb):
            for fn, c in zip(qk_group_closures(wq_sb, qT_sb, 1, tb), qk_costs):
                add_task(61 + 16 * tb, c, fn)

        pending = deque()
        BUDGET = 640.0
        O_COST = 450.0
        PT_MAX = 20  # pt_pool bufs - 2

        def head_eligible():
            return bool(pending) and v_ready[chunk_list[pending[0][0]][2]]

        def pop_one():
            j, jpt = pending.popleft()
            emit_O(j, jpt)

        for i in range(n_total):
            cur[0] = i
            ps, pt = emit_S_ACT(i)
            pending.append((i, pt))
            # overdue hard tasks run regardless of budget
            while task_q and task_q[0][0] <= i:
                _, _, _, fn = heapq.heappop(task_q)
                fn()
            # keep the pt backlog inside the pool
            while len(pending) > PT_MAX and head_eligible():
                pop_one()
            budget = BUDGET
            while budget > 0:
                if task_q and task_q[0][0] <= i + 2:
                    _, _, c, fn = heapq.heappop(task_q)
                    fn()
                    budget -= c
                elif len(pending) > 2 and head_eligible():
                    pop_one()
                    budget -= O_COST
                elif task_q:
                    _, _, c, fn = heapq.heappop(task_q)
                    fn()
                    budget -= c
                else:
                    break

        # drain phase: everything left, V-gated pops first-come-first-served
        cur[0] = n_total
        guard = 0
        while pending or task_q:
            guard += 1
            assert guard < 10000, "scheduler wedged"
            if head_eligible():
                pop_one()
            elif task_q:
                _, _, _, fn = heapq.heappop(task_q)
                fn()
            else:
                assert not pending, "pending O with no V path"

    nc.compile()
    return nc


def make_core_inputs(x, wq, wk, wv, wo):
    """Shard + pre-layout the full inputs into 8 per-core input maps."""
    bf = ml_dtypes.bfloat16
    in_maps = []
    for core in range(N_CORES):
        b = core // 4
        g = core % 4
        lo, hi = g * DH, (g + 1) * DH
        in_maps.append(
            {
                "xT": np.ascontiguousarray(x[b].T).astype(bf),
                "wqT": np.ascontiguousarray(wq[lo:hi, :].T).astype(bf),
                "wkT": np.ascontiguousarray(wk[lo:hi, :].T).astype(bf),
                "wvT": np.ascontiguousarray(wv[lo:hi, :].T).astype(bf),
                "woT": np.ascontiguousarray(wo[:, lo:hi].T).astype(bf),
            }
        )
    return in_maps


_PROGRAM_CACHE = {}


def _get_program():
    if "nc" not in _PROGRAM_CACHE:
        nc = build_program()
        nc.m = get_hw_module(nc.m)
        _PROGRAM_CACHE["nc"] = nc
    return _PROGRAM_CACHE["nc"]


def run_sharded(in_maps, trace=False):
    nc = _get_program()
    return bass_utils.run_bass_kernel_spmd(
        nc, in_maps, core_ids=list(range(N_CORES)), trace=trace
    )


def kernel(x, wq, wk, wv, wo):
    x = np.asarray(x, dtype=np.float32)
    wq = np.asarray(wq, dtype=np.float32)
    wk = np.asarray(wk, dtype=np.float32)
    wv = np.asarray(wv, dtype=np.float32)
    wo = np.asarray(wo, dtype=np.float32)

    in_maps = make_core_inputs(x, wq, wk, wv, wo)
    res = run_sharded(in_maps)

    B, T, C = x.shape
    out = np.zeros((B, T, C), dtype=np.float32)
    for core in range(N_CORES):
        out[core // 4] += res.results[core]["y"].astype(np.float32)
    return out


if __name__ == "__main__":
    rng = np.random.default_rng(0)
    x = rng.standard_normal((2, 2048, 1024), dtype=np.float32)
    s = 1.0 / np.sqrt(N_EMBD)
    ws = [rng.standard_normal((1024, 1024), dtype=np.float32) * s for _ in range(4)]
    out = kernel(x, *ws)
    print("out", out.shape, out.dtype, float(np.abs(out).max()))


# revision 34
# speedup vs baseline: 1.0227x; 1.0083x over previous
"""Multi-head attention kernel for 8 TRN2 NeuronCores (Bass/Tile).

Problem: x[2,2048,1024], 16 heads x 64 dims, torch-style Linear weights.
Sharding: data parallel over batch (2) x tensor parallel over heads (16/4):
core c handles batch c//4, heads 4*(c%4) .. 4*(c%4)+3. Each core computes
its heads' attention output projected through its slice of wo, producing a
partial [2048, 1024] fp16 output; the host sums the 4 partials per batch
(the "all-reduce after wo").

Device dataflow per core (matmul operands bf16, fp32 accumulation):
  QT/KT = weight-slice projections in [d, t] layout (d on partitions)
  V     = projection in natural [s, d] layout, with a ones column appended
          per head so the P@V matmul also yields the softmax denominator
  S^T   = K^T.T @ Q^T per head ([s, t] layout, s on partitions)
  P^T   = exp(S^T / 8) via ScalarE (no max subtraction: logits are O(8))
  O^T   = V.T @ P^T accumulated over s in PSUM (row 64 = denominator)
  y     = (O^T / denom)^T @ wo-slice^T, partial over this core's heads

The kernel is PE-bound at ~94% TensorE occupancy (~880 matmuls at
~215ns issue pitch); the 128-ACTIVATE exp train (~1.11us each) runs
inside that envelope.  Structure:
  - the exp table set is preloaded via a dummy ACTIVATE at t=0
  - input DMA order is wk, wq, x-first-half, wv, x-second-half, wo; the
    chase computes K-chunk0-g0 + Q(tb0) against the landing chunks and
    the ACT train starts as soon as the first half of x is projected
    (slots 0-7 only touch s<1024)
  - everything else (V, remaining K/Q groups, normalization, final wo
    matmuls, y DMAs) is emitted by a build-time budget scheduler:
    EDF-ordered micro-closures fill each slot's spare PE time, and the
    P@V consumers lag elastically behind the ACT train (deep pt
    buffering) gated on their V tile being ready
  - y is written as fp16 partials (summed on host), staged per 128-row
    block and DMA'd with 2KB descriptors as soon as each block's casts
    land; tail copies for the last pair run on the otherwise-idle
    ScalarE

HW-validated pitfalls baked into this design: a PSUM accumulation group
owns its whole 2KB bank (never interleave two groups in one bank); DVE
ops reading PSUM operands directly (reciprocal/tensor_mul) silently
corrupt on HW though CoreSim accepts them -- always stage through SBUF.
"""

import sys

sys.path.insert(0, "/opt/trn_rl_repo")

from contextlib import ExitStack

import ml_dtypes
import numpy as np

import concourse.bass as bass
import concourse.tile as tile
from concourse import bacc, mybir
from concourse import bass_utils
from concourse.bass_interp import get_hw_module

BF16 = mybir.dt.bfloat16
F16 = mybir.dt.float16
F32 = mybir.dt.float32
EXP = mybir.ActivationFunctionType.Exp

N_EMBD = 1024
N_HEAD = 16
HEAD_DIM = 64

N_CORES = 8
HEADS_PER_CORE = 4
DH = HEADS_PER_CORE * HEAD_DIM  # 256


def build_program(T=2048, C=N_EMBD, enable_asserts=False):
    nc = bacc.Bacc(
        "TRN2", target_bir_lowering=False, debug=False, enable_asserts=enable_asserts
    )

    xT = nc.dram_tensor("xT", [C, T], BF16, kind="ExternalInput").ap()
    wqT = nc.dram_tensor("wqT", [C, DH], BF16, kind="ExternalInput").ap()
    wkT = nc.dram_tensor("wkT", [C, DH], BF16, kind="ExternalInput").ap()
    wvT = nc.dram_tensor("wvT", [C, DH], BF16, kind="ExternalInput").ap()
    woT = nc.dram_tensor("woT", [DH, C], BF16, kind="ExternalInput").ap()
    y = nc.dram_tensor("y", [T, C], F16, kind="ExternalOutput").ap()

    n_ct = C // 128   # contraction tiles over embedding dim
    n_st = T // 128   # s tiles (key/value positions)
    n_tb = T // 512   # query blocks
    n_cb = C // 512   # output column blocks

    scale = float(HEAD_DIM**-0.5)

    with tile.TileContext(nc) as tc, ExitStack() as ctx:
        statics = ctx.enter_context(tc.tile_pool(name="statics", bufs=1))
        pt_pool = ctx.enter_context(tc.tile_pool(name="pt", bufs=22))
        onorm_pool = ctx.enter_context(tc.tile_pool(name="onorm", bufs=6))
        small = ctx.enter_context(tc.tile_pool(name="small", bufs=6))
        out_stage = ctx.enter_context(tc.tile_pool(name="out_stage", bufs=4))

        psum_s = ctx.enter_context(tc.tile_pool(name="psum_s", bufs=2, space="PSUM"))
        psum_o = ctx.enter_context(tc.tile_pool(name="psum_o", bufs=2, space="PSUM"))
        psum_f = ctx.enter_context(tc.tile_pool(name="psum_f", bufs=2, space="PSUM"))

        # ---- static SBUF tensors ----
        xT_sb = statics.tile([128, n_ct, T], BF16)
        wq_sb = statics.tile([128, n_ct, DH], BF16)
        wk_sb = statics.tile([128, n_ct, DH], BF16)
        wv_sb = statics.tile([128, n_ct, DH], BF16)
        wo_sb = statics.tile([128, 2, C], BF16)
        qT_sb = statics.tile([128, 2, T], BF16)
        kT_sb = statics.tile([128, 2, T], BF16)
        v_sb = statics.tile([128, n_st, HEADS_PER_CORE, HEAD_DIM + 1], BF16)
        oT_sb = statics.tile([128, 2, T], BF16)
        ones_sb = statics.tile([1, 64], BF16)
        warm_in = statics.tile([1, 64], F32)
        warm_out = statics.tile([1, 64], F32)

        # preload the exp table set while DMAs run (saves ~2.7us later)
        nc.vector.memset(warm_in, 0.0)
        nc.scalar.activation(warm_out, warm_in, EXP)

        xT_chunked = xT.rearrange("(a p) t -> p a t", p=128)
        half = T // 2
        wk_dram = wkT.rearrange("(a p) d -> p a d", p=128)
        wq_dram = wqT.rearrange("(a p) d -> p a d", p=128)
        nc.sync.dma_start(out=wk_sb[:, :, 0:128], in_=wk_dram[:, :, 0:128])
        nc.sync.dma_start(out=wq_sb[:, :, 0:128], in_=wq_dram[:, :, 0:128])
        for ct in range(n_ct):
            nc.sync.dma_start(out=xT_sb[:, ct, 0:half], in_=xT_chunked[:, ct, 0:half])
        nc.sync.dma_start(out=wv_sb, in_=wvT.rearrange("(a p) d -> p a d", p=128))
        for ct in range(n_ct):
            nc.sync.dma_start(out=xT_sb[:, ct, half:T], in_=xT_chunked[:, ct, half:T])
        nc.sync.dma_start(out=wk_sb[:, :, 128:256], in_=wk_dram[:, :, 128:256])
        nc.sync.dma_start(out=wq_sb[:, :, 128:256], in_=wq_dram[:, :, 128:256])
        nc.sync.dma_start(out=wo_sb, in_=woT.rearrange("(a p) c -> p a c", p=128))
        nc.vector.memset(ones_sb, 1.0)
        nc.vector.memset(v_sb[:, :, :, HEAD_DIM : HEAD_DIM + 1], 1.0)

        # ---- startup chase: K chunk0 groups 0/1 (s 0-1023) + Q chunk0 tb0,
        # accumulated ct-outer chasing the first-half x chunks as they land.
        # The ACT train starts right after: slots 0-7 only touch s<1024.
        # K g2/g3, Q tb1 and all of V are scheduler tasks (O-pops are
        # elastic, so V tiles may trail their s-slot by many slots).
        proj_scope = nc.named_scope("proj")
        proj_scope.__enter__()

        kps0 = psum_s.tile([128, 1024], F32, tag="s")
        qps0 = psum_f.tile([128, 512], F32, tag="f")
        # K g0 (DMA-paced) then Q tb0; their copies run on ScalarE/DVE in
        # parallel, so S(0) issues ~0.7us after the Q loop ends.  K g1
        # (first read at slot 4) is deferred into the early slot budget.
        for ct in range(n_ct):
            for g in range(2):
                nc.tensor.matmul(
                    kps0[:, g * 512 : (g + 1) * 512],
                    lhsT=wk_sb[:, ct, 0:128],
                    rhs=xT_sb[:, ct, g * 512 : (g + 1) * 512],
                    start=(ct == 0),
                    stop=(ct == n_ct - 1),
                )
            nc.tensor.matmul(
                qps0,
                lhsT=wq_sb[:, ct, 0:128],
                rhs=xT_sb[:, ct, 0:512],
                start=(ct == 0),
                stop=(ct == n_ct - 1),
            )
        nc.scalar.copy(kT_sb[:, 0, 0:512], kps0[:, 0:512])
        nc.vector.tensor_copy(qT_sb[:, 0, 0:512], qps0)
        nc.vector.tensor_copy(kT_sb[:, 0, 512:1024], kps0[:, 512:1024])

        proj_scope.__exit__(None, None, None)

        # ---- attention: software-pipelined head-pair chunk loop ----
        # One slot = one s-tile for a pair of heads (hp): the two K=64 score
        # matmuls land on disjoint PE row groups (base partitions 0 and 64)
        # and run concurrently; both scores share one [128,1024] PSUM tile so
        # a single ACTIVATE computes exp for both heads.
        chunk_list = []
        for hp in range(HEADS_PER_CORE // 2):
            for tb in range(n_tb):
                for c in range(n_st):
                    chunk_list.append((tb, hp, c))
        n_total = len(chunk_list)

        o_ps_map = {}
        denom_map = {}
        o_unnorm_map = {}

        def emit_S_ACT(i):
            tb, hp, st = chunk_list[i]
            chunk_hd = hp
            if st == 0:
                o_psA = psum_o.tile([HEAD_DIM + 1, 512], F32, tag="o")
                o_psB = psum_o.tile([HEAD_DIM + 1, 512], F32, tag="o")
                o_ps_map[(tb, 2 * hp)] = o_psA
                o_ps_map[(tb, 2 * hp + 1)] = o_psB
            with nc.named_scope("S"):
                ps = psum_s.tile([128, 2 * 512], F32, tag="s")
                for half_i, dlo in ((0, 0), (1, 64)):
                    nc.tensor.matmul(
                        ps[:, half_i * 512 : (half_i + 1) * 512],
                        lhsT=kT_sb[dlo : dlo + 64, chunk_hd, st * 128 : (st + 1) * 128],
                        rhs=qT_sb[dlo : dlo + 64, chunk_hd, tb * 512 : (tb + 1) * 512],
                        start=True,
                        stop=True,
                        tile_position=(dlo, 0),
                    )
            with nc.named_scope("exp"):
                pt = pt_pool.tile([128, 2, 512], BF16)
                nc.scalar.activation(
                    pt.rearrange("p a b -> p (a b)"), ps, EXP, scale=scale
                )
            return ps, pt

        def emit_O(i, pt):
            tb, hp, st = chunk_list[i]
            with nc.named_scope("O"):
                for half_i in range(2):
                    h = 2 * hp + half_i
                    nc.tensor.matmul(
                        o_ps_map[(tb, h)],
                        lhsT=v_sb[:, st, h, :],
                        rhs=pt[:, half_i, :],
                        start=(st == 0),
                        stop=(st == n_st - 1),
                    )
            if st == n_st - 1:
                # pair (tb, hp) fully accumulated: drain PSUM immediately so
                # the banks recycle for the next pair (unnormalized O + denom)
                tail = hp == 1 and tb == 3
                for half_i in range(2):
                    h = 2 * hp + half_i
                    o_ps = o_ps_map[(tb, h)]
                    o_unnorm = onorm_pool.tile([64, 512], BF16, tag="ou")
                    if tail and half_i == 1:
                        nc.scalar.copy(o_unnorm, o_ps[0:64, :])
                        denom_f = small.tile([1, 512], F32, tag="denom_f")
                        nc.scalar.copy(denom_f, o_ps[64:65, :])
                    else:
                        nc.vector.tensor_copy(o_unnorm, o_ps[0:64, :])
                        denom_f = small.tile([1, 512], F32, tag="denom_f")
                        nc.vector.tensor_copy(denom_f, o_ps[64:65, :])
                    o_unnorm_map[(tb, h)] = o_unnorm
                    denom_map[(tb, h)] = denom_f
                for half_i in range(2):
                    h = 2 * hp + half_i
                    add_task(
                        cur[0] + 7 + half_i, 300.0,
                        (lambda tb=tb, h=h: run_norm(tb, h)),
                    )

        def emit_norm(tb, h):
            ns = nc.named_scope("norm"); ns.__enter__()
            tail = tb == 3 and h >= 2
            chunk_hd, dlo = h // 2, (h % 2) * 64
            recip_f = small.tile([1, 512], F32, tag="recip_f")
            nc.vector.reciprocal_approx_fast(recip_f, denom_map[(tb, h)])
            recip = small.tile([1, 512], BF16, tag="recip")
            if tail:
                nc.scalar.copy(recip, recip_f)
            else:
                nc.vector.tensor_copy(recip, recip_f)
            rep = psum_f.tile([128, 512], F32, tag="f")
            nc.tensor.matmul(
                rep[0:64, :], lhsT=ones_sb, rhs=recip, start=True, stop=True
            )
            rep_sb = small.tile([64, 512], BF16, tag="rep")
            if tail:
                nc.scalar.copy(rep_sb, rep[0:64, :])
            else:
                nc.vector.tensor_copy(rep_sb, rep[0:64, :])
            nc.vector.tensor_mul(
                oT_sb[dlo : dlo + 64, chunk_hd, tb * 512 : (tb + 1) * 512],
                o_unnorm_map[(tb, h)],
                rep_sb,
            )
            ns.__exit__(None, None, None)

        def qk_group_closures(w_sb, dst, chunk, tb):
            cell = {}

            def mk_mm(c0):
                def f():
                    if "ps" not in cell:
                        ps_qk = psum_f.tile([128, 512], F32, tag="f")
                        cell["ps"] = ps_qk
                    ps_qk = cell["ps"]
                    for ct in range(c0, c0 + 2):
                        nc.tensor.matmul(
                            ps_qk,
                            lhsT=w_sb[:, ct, chunk * 128 : (chunk + 1) * 128],
                            rhs=xT_sb[:, ct, tb * 512 : (tb + 1) * 512],
                            start=(ct == 0),
                            stop=(ct == n_ct - 1),
                        )
                return f

            def cp():
                nc.vector.tensor_copy(
                    dst[:, chunk, tb * 512 : (tb + 1) * 512], cell["ps"]
                )

            return [mk_mm(0), mk_mm(2), mk_mm(4), mk_mm(6), cp]

        def v_pair_closures(p):
            """s-tiles 2p, 2p+1 computed SEQUENTIALLY through one psum bank
            (a PSUM accumulation group owns its whole bank, so the two
            s-tiles' groups must not interleave)."""
            cell = {}

            def mk(k, c0):
                def f():
                    if k != cell.get("k"):
                        ps_v_new = psum_f.tile([128, 512], F32, tag="f")
                        cell["ps"] = ps_v_new
                        cell["k"] = k
                    ps_v = cell["ps"]
                    st = 2 * p + k
                    for ct in range(c0, c0 + 4):
                        nc.tensor.matmul(
                            ps_v[:, 0:DH],
                            lhsT=xT_sb[:, ct, st * 128 : (st + 1) * 128],
                            rhs=wv_sb[:, ct, :],
                            start=(ct == 0),
                            stop=(ct == n_ct - 1),
                        )
                return f

            def cp(k):
                def f():
                    st = 2 * p + k
                    nc.vector.tensor_copy(
                        v_sb[:, st, :, 0:HEAD_DIM],
                        cell["ps"][:, 0:DH].rearrange(
                            "p (h d) -> p h d", h=HEADS_PER_CORE
                        ),
                    )
                return f

            return [mk(0, 0), mk(0, 4), cp(0), mk(1, 0), mk(1, 4), cp(1)]

        def final_closures(tb):
            # per-tt staging: each 128-row block CASTs into its own staged
            # [128, 1024] fp16 tile and DMAs out as soon as both halves have
            # landed (2KB descriptors; issued early so the write flight
            # overlaps the remaining compute).  For the last-processed block
            # (tb3) the casts alternate onto the idle ScalarE so the tail
            # is not DVE-serial.
            y_blocked = y.rearrange("(b t p) c -> p b t c", b=n_tb, p=128)
            tail = tb == 3
            cells = {}
            out = []
            for tt4 in range(4):
                for cb in range(n_cb):
                    def f(tt4=tt4, cb=cb):
                        ns = nc.named_scope("final")
                        ns.__enter__()
                        if tt4 not in cells:
                            y_st = out_stage.tile([128, C], F16)
                            cells[tt4] = y_st
                        tt = tb * 4 + tt4
                        ps_fin = psum_f.tile([128, 512], F32, tag="f")
                        for chunk in range(2):
                            nc.tensor.matmul(
                                ps_fin,
                                lhsT=oT_sb[:, chunk, tt * 128 : (tt + 1) * 128],
                                rhs=wo_sb[:, chunk, cb * 512 : (cb + 1) * 512],
                                start=(chunk == 0),
                                stop=(chunk == 1),
                            )
                        dst = cells[tt4][:, cb * 512 : (cb + 1) * 512]
                        if tail and cb == 1:
                            nc.scalar.copy(dst, ps_fin)
                        else:
                            nc.vector.tensor_copy(dst, ps_fin)
                        ns.__exit__(None, None, None)

                    out.append(f)

                def dma(tt4=tt4):
                    nc.sync.dma_start(
                        out=y_blocked[:, tb, tt4, :], in_=cells[tt4]
                    )

                out.append(dma)
            return out

        # ---------- build-time budget scheduler ----------
        # The 128-slot ACT train is the pacer (~1.11us/slot).  Each slot
        # emits its S+ACT, then fills ~BUDGET ns of PE work: overdue
        # hard-deadline tasks first, then O-pops (elastic backlog, gated on
        # the V tile they consume) and EDF-ordered deferred closures.
        import heapq
        from collections import deque

        task_q = []  # (deadline, seq, cost_ns, fn)
        _seq = [0]

        def add_task(deadline, cost, fn):
            heapq.heappush(task_q, (deadline, _seq[0], cost, fn))
            _seq[0] += 1

        v_ready = [False] * n_st
        norm_cnt = {}
        cur = [0]

        def run_norm(tb, h):
            emit_norm(tb, h)
            norm_cnt[tb] = norm_cnt.get(tb, 0) + 1
            if norm_cnt[tb] == HEADS_PER_CORE:
                for k, fn in enumerate(final_closures(tb)):
                    add_task(cur[0] + 1 + k, 520.0, fn)

        # V pairs p=0..7; the cp closures flip v_ready for their s-tile
        for p in range(n_st // 2):
            cl = v_pair_closures(p)
            costs = [450.0, 450.0, 80.0, 450.0, 450.0, 80.0]

            def flag(fn, st):
                def g():
                    fn()
                    v_ready[st] = True
                return g

            cl[2] = flag(cl[2], 2 * p)
            cl[5] = flag(cl[5], 2 * p + 1)
            for fn, c in zip(cl, costs):
                add_task(max(1, 2 * p + 10), c, fn)

        qk_costs = [440.0, 440.0, 440.0, 440.0, 80.0]
        # second-half K chunk0 groups + Q chunk0 tb1-3 (hard deadlines:
        # first read at slots 8/12 and 16/32/48)
        for fn, c in zip(qk_group_closures(wk_sb, kT_sb, 0, 2), qk_costs):
            add_task(6, c, fn)
        for fn, c in zip(qk_group_closures(wk_sb, kT_sb, 0, 3), qk_costs):
            add_task(10, c, fn)
        for fn, c in zip(qk_group_closures(wq_sb, qT_sb, 0, 1), qk_costs):
            add_task(14, c, fn)
        for fn, c in zip(qk_group_closures(wq_sb, qT_sb, 0, 2), qk_costs):
            add_task(29, c, fn)
        for fn, c in zip(qk_group_closures(wq_sb, qT_sb, 0, 3), qk_costs):
            add_task(45, c, fn)
        for g in range(n_tb):
            for fn, c in zip(qk_group_closures(wk_sb, kT_sb, 1, g), qk_costs):
                add_task(58 + 4 * g, c, fn)
        for tb in range(n_tb):
            for fn, c in zip(qk_group_closures(wq_sb, qT_sb, 1, tb), qk_costs):
                add_task(61 + 16 * tb, c, fn)

        pending = deque()
        BUDGET = 620.0
        O_COST = 450.0
        PT_MAX = 20  # pt_pool bufs - 2

        def head_eligible():
            return bool(pending) and v_ready[chunk_list[pending[0][0]][2]]

        def pop_one():
            j, jpt = pending.popleft()
            emit_O(j, jpt)

        for i in range(n_total):
            cur[0] = i
            ps, pt = emit_S_ACT(i)
            pending.append((i, pt))
            # overdue hard tasks run regardless of budget
            while task_q and task_q[0][0] <= i:
                _, _, _, fn = heapq.heappop(task_q)
                fn()
            # keep the pt backlog inside the pool
            while len(pending) > PT_MAX and head_eligible():
                pop_one()
            budget = BUDGET
            while budget > 0:
                if task_q and task_q[0][0] <= i + 2:
                    _, _, c, fn = heapq.heappop(task_q)
                    fn()
                    budget -= c
                elif len(pending) > 2 and head_eligible():
                    pop_one()
                    budget -= O_COST
                elif task_q:
                    _, _, c, fn = heapq.heappop(task_q)
                    fn()
                    budget -= c
                else:
                    break

        # drain phase: everything left, V-gated pops first-come-first-served
        cur[0] = n_total
        guard = 0
        while pending or task_q:
            guard += 1
            assert guard < 10000, "scheduler wedged"
            if head_eligible():
                pop_one()
            elif task_q:
                _, _, _, fn = heapq.heappop(task_q)
                fn()
            else:
                assert not pending, "pending O with no V path"

    nc.compile()
    return nc


def make_core_inputs(x, wq, wk, wv, wo):
    """Shard + pre-layout the full inputs into 8 per-core input maps."""
    bf = ml_dtypes.bfloat16
    in_maps = []
    for core in range(N_CORES):
        b = core // 4
        g = core % 4
        lo, hi = g * DH, (g + 1) * DH
        in_maps.append(
            {
                "xT": np.ascontiguousarray(x[b].T).astype(bf),
                "wqT": np.ascontiguousarray(wq[lo:hi, :].T).astype(bf),
                "wkT": np.ascontiguousarray(wk[lo:hi, :].T).astype(bf),
                "wvT": np.ascontiguousarray(wv[lo:hi, :].T).astype(bf),
                "woT": np.ascontiguousarray(wo[:, lo:hi].T).astype(bf),
            }
        )
    return in_maps


_PROGRAM_CACHE = {}


def _get_program():
    if "nc" not in _PROGRAM_CACHE:
        nc = build_program()
        nc.m = get_hw_module(nc.m)
        _PROGRAM_CACHE["nc"] = nc
    return _PROGRAM_CACHE["nc"]


def run_sharded(in_maps, trace=False):
    nc = _get_program()
    return bass_utils.run_bass_kernel_spmd(
        nc, in_maps, core_ids=list(range(N_CORES)), trace=trace
    )


def kernel(x, wq, wk, wv, wo):
    x = np.asarray(x, dtype=np.float32)
    wq = np.asarray(wq, dtype=np.float32)
    wk = np.asarray(wk, dtype=np.float32)
    wv = np.asarray(wv, dtype=np.float32)
    wo = np.asarray(wo, dtype=np.float32)

    in_maps = make_core_inputs(x, wq, wk, wv, wo)
    res = run_sharded(in_maps)

    B, T, C = x.shape
    out = np.zeros((B, T, C), dtype=np.float32)
    for core in range(N_CORES):
        out[core // 4] += res.results[core]["y"].astype(np.float32)
    return out


if __name__ == "__main__":
    rng = np.random.default_rng(0)
    x = rng.standard_normal((2, 2048, 1024), dtype=np.float32)
    s = 1.0 / np.sqrt(N_EMBD)
    ws = [rng.standard_normal((1024, 1024), dtype=np.float32) * s for _ in range(4)]
    out = kernel(x, *ws)
    print("out", out.shape, out.dtype, float(np.abs(out).max()))
